# revision 4
# baseline (speedup 1.0000x reference)
"""DGMRF 2-layer GNN message passing on 8 TRN2 NeuronCores.

Strategy (per layer, per core):
  - Nodes sharded by dst: core c owns dst nodes [12500c, 12500(c+1)).
  - Aggregation aggr[:, d] = sum_{e: dst=d} x[:, src_e] done on-device via
    GPSIMD ap_gather (SBUF free-dim gather, per-16-partition-group index
    lists) + DVE tensor_reduce over per-dst slot grids.
  - x is chunked into 4 source chunks of 25000 nodes (int16 index limit);
    per chunk, per exact-degree class K, a padded [Npad, K] grid of
    chunk-local src indices is gathered and reduced; per-chunk results are
    reordered into natural dst order with a second (small) ap_gather and
    accumulated.
  - Per-node factors f1/f2 (degree weights x layer scalars) are computed on
    host; out = f1*x_self + f2*aggr + bias.
  - One compiled SPMD program, run once per layer; host re-assembles the
    full x between layers (the cross-core "halo exchange").
"""
import numpy as np

import concourse.bass as bass
import concourse.mybir as mybir
import concourse.tile as tile
from concourse import bacc
from concourse.bass_utils import run_bass_kernel_spmd

N_NODES = 100000
N_CORES = 8
NPC = N_NODES // N_CORES          # 12500 dst nodes per core
G = 8                             # partition groups per core
GW_REAL = 1563                    # dst nodes per group (last group short)
GW = 1568                         # padded group width (mult of 16 for idx wrap)
NPC_PAD = G * GW_REAL             # 12504
CHUNK = 25000
N_CHUNKS = N_NODES // CHUNK       # 4
TBL = CHUNK + 1                   # table width incl zero dummy col
DUMMY = CHUNK                     # dummy (zero) column index


def _round_up(x, m):
    return (x + m - 1) // m * m


def _node_group(j):
    """group id for core-local padded dst position j (0..NPC_PAD)."""
    return np.minimum(j // GW_REAL, G - 1)


def _wrap_idx(flat):
    """[G, Q] flat per-group index lists -> wrapped [128, Q//16] int16.

    ap_gather consumes per-group lists interleaved: list position q lives at
    partition 16g + q%16, column q//16."""
    Gn, Q = flat.shape
    assert Q % 16 == 0
    w = flat.reshape(Gn, Q // 16, 16).transpose(0, 2, 1)  # [G, 16, Q//16]
    return w.reshape(Gn * 16, Q // 16).astype(np.int16)


def make_plan(edge_index):
    """Host-side planning: class structure (global) + per-core index data."""
    src = np.asarray(edge_index[0], dtype=np.int64)
    dst = np.asarray(edge_index[1], dtype=np.int64)
    E = src.shape[0]

    chunk_of = src // CHUNK
    # per (dst, chunk) degree
    cnt = np.bincount(dst * N_CHUNKS + chunk_of, minlength=N_NODES * N_CHUNKS)
    cnt = cnt.reshape(N_NODES, N_CHUNKS)

    # edges sorted by (dst, chunk): per (dst, chunk) contiguous src runs
    order = np.lexsort((chunk_of, dst))
    s_src_local = (src[order] % CHUNK).astype(np.int32)
    starts = np.zeros(N_NODES * N_CHUNKS + 1, np.int64)
    np.cumsum(cnt.reshape(-1), out=starts[1:])

    # core-local padded dst table: pos (c, j) -> node id or -1 (pad)
    # groups are contiguous slices of width GW_REAL
    core_node = np.full((N_CORES, NPC_PAD), -1, np.int64)
    for c in range(N_CORES):
        core_node[c, :NPC] = np.arange(NPC * c, NPC * (c + 1))
    grp_of_j = _node_group(np.arange(NPC_PAD))

    # per (chunk): class structure, global across cores/groups
    plan = []
    SW = 0  # total wrapped idx columns
    per_core_idx = [[] for _ in range(N_CORES)]  # list of wrapped [128, S] arrays

    for cc in range(N_CHUNKS):
        deg = np.zeros((N_CORES, NPC_PAD), np.int64)
        valid = core_node >= 0
        deg[valid] = cnt[core_node[valid], cc]

        kmax = int(deg.max())
        # global per-(core,group,K) counts
        counts = np.zeros((N_CORES, G, kmax + 1), np.int64)
        for c in range(N_CORES):
            key = grp_of_j * (kmax + 1) + deg[c]
            bc = np.bincount(key, minlength=G * (kmax + 1))
            counts[c] = bc.reshape(G, kmax + 1)
        gmax = counts.max(axis=(0, 1))  # [kmax+1] max members per group

        # kept classes: merge small ones upward
        kept = []
        for K in range(1, kmax + 1):
            if gmax[K] == 0:
                continue
            kept.append(K)
        # merge: keep K if its max count >= 48, else merge into next kept
        merged = []
        for K in kept:
            merged.append(K)
        # assignment class of each deg d (>0): smallest kept >= d
        keep_mask = [False] * (kmax + 2)
        sel = []
        for K in kept:
            if gmax[K] >= 48 or K == kept[-1]:
                sel.append(K)
        if not sel or sel[-1] < kmax:
            sel.append(kmax)
        sel = sorted(set(sel))
        cls_of = np.zeros(kmax + 1, np.int64)  # deg -> class K (0 for deg 0)
        si = 0
        for d in range(1, kmax + 1):
            while sel[si] < d:
                si += 1
            cls_of[d] = sel[si]

        # per class: Npad (global), temp offset
        cls_list = []
        off = 0
        npad_of = {}
        for K in sel:
            mx = 0
            for c in range(N_CORES):
                cdeg = deg[c]
                mask = (cdeg > 0) & (cls_of[cdeg] == K)
                bc = np.bincount(grp_of_j[mask], minlength=G)
                mx = max(mx, int(bc.max()))
            from math import gcd
            # npad*K must be a multiple of 32 so every idx slice starts at an
            # even int16 column (Q7 reads index lists as packed uint32s)
            m = 32 // gcd(K, 32)
            npad = _round_up(mx + 1, max(m, 4))  # +1 guarantees a zero pad cell
            npad_of[K] = (npad, off)
            cls_list.append((K, npad, off))
            off += npad
        WT = off
        zero_cell = cls_list[0][2] + cls_list[0][1] - 1  # a guaranteed 0 cell

        # build idx data per core
        chunk_cols = 0
        for c in range(N_CORES):
            cdeg = deg[c]
            pieces = []
            # rank within (group, class)
            rank = np.zeros(NPC_PAD, np.int64)
            tempcol = np.full(NPC_PAD, zero_cell, np.int64)
            for K, npad, offK in cls_list:
                A = np.full((G, npad, K), DUMMY, np.int32)
                mask = (cdeg > 0) & (cls_of[cdeg] == K)
                js = np.nonzero(mask)[0]
                gs = grp_of_j[js]
                # rank via cumcount per group (js ascending => natural order)
                r = np.zeros(len(js), np.int64)
                for g in range(G):
                    m = gs == g
                    r[m] = np.arange(m.sum())
                rank[js] = r
                tempcol[js] = offK + r
                # fill srcs: node n, chunk cc: run starts[n*4+cc], length deg
                nodes = core_node[c, js]
                st = starts[nodes * N_CHUNKS + cc]
                dg = cdeg[js]
                for i in range(len(js)):
                    A[gs[i], r[i], :dg[i]] = s_src_local[st[i]:st[i] + dg[i]]
                pieces.append(_wrap_idx(A.reshape(G, npad * K)))
            # reorder idx: [G, GW] -> temp col of each natural dst position
            R = np.full((G, GW), zero_cell, np.int32)
            for g in range(G):
                jj = np.arange(GW_REAL) + g * GW_REAL
                R[g, :GW_REAL] = tempcol[jj]
            pieces.append(_wrap_idx(R))
            blob = np.concatenate(pieces, axis=1)
            per_core_idx[c].append(blob)
            chunk_cols = blob.shape[1]

        # column offsets within the chunk blob
        coffs = []
        co = 0
        for K, npad, offK in cls_list:
            coffs.append((K, npad, offK, co, npad * K // 16))
            co += npad * K // 16
        reorder_off = co
        co += GW // 16
        assert co == chunk_cols
        plan.append(dict(classes=coffs, WT=WT, reorder_off=reorder_off,
                         cols=chunk_cols, col_base=SW))
        SW += chunk_cols

    idx_all = []
    for c in range(N_CORES):
        idx_all.append(np.concatenate(per_core_idx[c], axis=1))
        assert idx_all[c].shape == (128, SW)
    return plan, idx_all, SW


def build_program(plan, SW):
    nc = bacc.Bacc("TRN2", target_bir_lowering=False, debug=False, num_devices=1)
    f32 = mybir.dt.float32
    xin = nc.dram_tensor("xin", [16, N_NODES], f32, kind="ExternalInput")
    xself = nc.dram_tensor("xself", [16, G * GW], f32, kind="ExternalInput")
    idxin = nc.dram_tensor("idxall", [128, SW], mybir.dt.int16, kind="ExternalInput")
    fin = nc.dram_tensor("fpp", [128, 3 * GW], f32, kind="ExternalInput")
    outd = nc.dram_tensor("out", [16, G * GW], f32, kind="ExternalOutput")

    WTmax = max(p["WT"] for p in plan)
    slab_max = max(npad * K for p in plan for (K, npad, _, _, _) in p["classes"])

    with tile.TileContext(nc) as tc:
        with (
            tc.tile_pool(name="tabp", bufs=1) as tabp,
            tc.tile_pool(name="fixp", bufs=1) as fixp,
            tc.tile_pool(name="slabp", bufs=3) as slabp,
            tc.tile_pool(name="rop", bufs=2) as rop,
        ):
            idxs = fixp.tile([128, SW], mybir.dt.int16)
            nc.sync.dma_start(out=idxs[:], in_=idxin[:])
            fbuf = fixp.tile([128, 3 * GW], f32)
            nc.sync.dma_start(out=fbuf[:], in_=fin[:])
            selfb = fixp.tile([128, GW], f32)
            for g in range(G):
                nc.sync.dma_start(out=selfb[16 * g:16 * (g + 1), :],
                                  in_=xself[:, GW * g:GW * (g + 1)])
            acc = fixp.tile([128, GW], f32)
            temp = fixp.tile([128, WTmax], f32)

            for cc in range(N_CHUNKS):
                table = tabp.tile([128, TBL], f32, tag="table")
                for g in range(G):
                    nc.sync.dma_start(
                        out=table[16 * g:16 * (g + 1), 0:CHUNK],
                        in_=xin[:, CHUNK * cc:CHUNK * (cc + 1)])
                nc.vector.memset(table[:, CHUNK:TBL], 0.0)
                p = plan[cc]
                nc.vector.memset(temp[:, :p["WT"]], 0.0)
                for (K, npad, offK, co, ncols) in p["classes"]:
                    slab = slabp.tile([128, slab_max], f32, tag="slab")
                    cbase = p["col_base"] + co
                    nc.gpsimd.ap_gather(
                        slab[:, :npad * K], table[:, :TBL],
                        idxs[:, cbase:cbase + ncols],
                        channels=128, num_elems=TBL, d=1, num_idxs=npad * K)
                    nc.vector.tensor_reduce(
                        temp[:, offK:offK + npad],
                        slab[:, :npad * K].rearrange("p (n k) -> p n k", k=K),
                        axis=mybir.AxisListType.X, op=mybir.AluOpType.add)
                ro = rop.tile([128, GW], f32, tag="ro")
                rbase = p["col_base"] + p["reorder_off"]
                nc.gpsimd.ap_gather(
                    ro[:], temp[:, :p["WT"]], idxs[:, rbase:rbase + GW // 16],
                    channels=128, num_elems=p["WT"], d=1, num_idxs=GW)
                if cc == 0:
                    nc.vector.tensor_copy(out=acc[:], in_=ro[:])
                else:
                    nc.vector.tensor_add(out=acc[:], in0=acc[:], in1=ro[:])

            outb = fixp.tile([128, GW], f32)
            nc.vector.tensor_mul(out=outb[:], in0=selfb[:], in1=fbuf[:, 0:GW])
            nc.vector.tensor_mul(out=acc[:], in0=acc[:], in1=fbuf[:, GW:2 * GW])
            nc.vector.tensor_add(out=outb[:], in0=outb[:], in1=acc[:])
            nc.vector.tensor_add(out=outb[:], in0=outb[:], in1=fbuf[:, 2 * GW:3 * GW])
            for g in range(G):
                nc.sync.dma_start(out=outd[:, GW * g:GW * (g + 1)],
                                  in_=outb[16 * g:16 * (g + 1), :])
    nc.compile()
    return nc


def _per_core_slices(xfull):
    """xfull [16, N] -> per-core xself [16, G*GW] group-major padded."""
    out = []
    for c in range(N_CORES):
        xs = np.zeros((16, G * GW), np.float32)
        for g in range(G):
            lo = NPC * c + GW_REAL * g
            sz = min(GW_REAL, NPC - GW_REAL * g)
            if sz > 0:
                xs[:, GW * g:GW * g + sz] = xfull[:, lo:lo + sz]
        out.append(xs)
    return out


def _assemble(outs):
    """per-core out [16, G*GW] -> xfull [16, N]."""
    xf = np.empty((16, N_NODES), np.float32)
    for c in range(N_CORES):
        o = outs[c]
        for g in range(G):
            lo = NPC * c + GW_REAL * g
            sz = min(GW_REAL, NPC - GW_REAL * g)
            if sz > 0:
                xf[:, lo:lo + sz] = o[:, GW * g:GW * g + sz]
    return xf


def _fpp(f1, f2, b):
    """per-node f1/f2 [N] + scalar bias -> per-core [128, 3*GW] tensors."""
    out = []
    for c in range(N_CORES):
        t = np.zeros((128, 3 * GW), np.float32)
        for g in range(G):
            lo = NPC * c + GW_REAL * g
            sz = min(GW_REAL, NPC - GW_REAL * g)
            rows = slice(16 * g, 16 * (g + 1))
            if sz > 0:
                t[rows, 0:sz] = np.broadcast_to(f1[lo:lo + sz], (16, sz))
                t[rows, GW:GW + sz] = np.broadcast_to(f2[lo:lo + sz], (16, sz))
            t[rows, 2 * GW:3 * GW] = b
        out.append(t)
    return out


_CACHE = {}


def kernel(x, edge_index, alpha1, gamma, bias):
    x = np.asarray(x, dtype=np.float32)
    edge_index = np.asarray(edge_index)
    alpha1 = np.asarray(alpha1, dtype=np.float64)
    gamma = np.asarray(gamma, dtype=np.float64)
    bias = np.asarray(bias, dtype=np.float64)
    n_layers = alpha1.shape[0]

    key = hash(edge_index.tobytes())
    if key not in _CACHE:
        plan, idx_all, SW = make_plan(edge_index)
        nc = build_program(plan, SW)
        _CACHE[key] = (plan, idx_all, SW, nc)
    plan, idx_all, SW, nc = _CACHE[key]

    src = np.asarray(edge_index[0], dtype=np.int64)
    deg = np.bincount(src, minlength=N_NODES).astype(np.float64)
    with np.errstate(divide="ignore"):
        logdeg = np.log(deg)

    xcur = x
    for l in range(n_layers):
        a1 = alpha1[l].reshape(())
        dp = 1.0 / (1.0 + np.exp(-gamma[l].reshape(())))
        self_w = np.exp(a1)
        neigh_w = self_w * np.tanh(a1)
        b = bias[l].reshape(())
        f1 = (self_w * np.exp(dp * logdeg)).astype(np.float32)
        f2 = (neigh_w * np.exp((dp - 1.0) * logdeg)).astype(np.float32)
        fs = _fpp(f1, f2, np.float32(b))
        xselfs = _per_core_slices(xcur)
        in_maps = [
            {"xin": np.ascontiguousarray(xcur), "xself": xselfs[c],
             "idxall": idx_all[c], "fpp": fs[c]}
            for c in range(N_CORES)
        ]
        res = run_bass_kernel_spmd(nc, in_maps, core_ids=list(range(N_CORES)))
        xcur = _assemble([r["out"] for r in res.results])
    return xcur
